# revision 16
# baseline (speedup 1.0000x reference)
"""Trainium2 Bass kernel for STSBaselineNet (embed -> biLSTM -> max-pool).

Sharding: one LSTM direction per core. Cores 0-3 run the forward pass of
sentence blocks 0-3; cores 4-7 run the backward pass of the same blocks
(time reversal and pad masking folded into host data prep: reversed token
order plus a -BIG pad-flag lane on the i/f/o logits).

v4 structure:
  - Token scan order is s-major so every PSUM->SBUF copy is contiguous.
  - The recurrence PSUM is split into three bank-aligned tiles
    [i,f | g | o], each preloaded with its zx slice by an identity
    -stationary matmul (start=True) that the W_hh pairs accumulate onto.
    Tile-granular dependency tracking then lets sigmoid(i,f) issue as
    soon as the i/f pairs stop, before the g/o matmuls finish.
  - Elementwise chain: sig(i,f) -> [c*=f | tmp=i*tanh(g)] -> c+=tmp ->
    tanh(c) -> h=o*tch, with the running masked max on GpSimd.
  - Phase A (gather / transpose / projection) is streamed into the
    recurrence as paced work items so the PE's elementwise stalls are
    filled with projection matmuls (keeps HAM at K=8/8).
"""

import numpy as np
import ml_dtypes

import concourse.bass as bass
import concourse.bacc as bacc
import concourse.mybir as mybir
import concourse.tile as tile
from concourse import bass_utils

V, E, HID, B, T = 50000, 300, 256, 256, 64
NCORES = 8
NSC = 64                    # sentences per core (one direction)
NTOK = NSC * T              # 4096 tokens/core
NTT = NTOK // 128           # 32 gather tiles
EP = 384                    # padded feature dim (300 emb + bias + flag + pad)
BIGNEG = -30.0              # logit offset for gate masking (bwd cores)
MAXNEG = -8.0               # mask offset for the final max (|h| < 1)

F32 = mybir.dt.float32
BF16 = mybir.dt.bfloat16
I32 = mybir.dt.int32
AF = mybir.ActivationFunctionType
OP = mybir.AluOpType

bf = ml_dtypes.bfloat16

# gate chunk order: [i i f f g g o o] == torch row order (256 rows each)
GB_BASE = {ch: ch * 128 for ch in range(8)}

_CACHE = {}
LAST_RESULTS = None


def _build_program():
    nc = bacc.Bacc(None, target_bir_lowering=False)

    emb_d = nc.dram_tensor("emb", [V, EP], BF16, kind="ExternalInput")
    idx_d = nc.dram_tensor("idx", [128, NTT], I32, kind="ExternalInput")
    mflag_d = nc.dram_tensor("mflag", [128, NTT], BF16, kind="ExternalInput")
    wstat_d = nc.dram_tensor("wstat", [128, 2048], BF16, kind="ExternalInput")
    wih_d = nc.dram_tensor("wih", [128, 3072], BF16, kind="ExternalInput")
    mbig_d = nc.dram_tensor("mbig", [128, 8192], BF16, kind="ExternalInput")
    out_d = nc.dram_tensor("out", [NSC, HID], F32, kind="ExternalOutput")

    with tile.TileContext(nc) as tc:
        with (
            tc.tile_pool(name="const", bufs=1) as cpool,
            tc.tile_pool(name="psumz", bufs=1, space="PSUM") as zpool,
            tc.tile_pool(name="psump", bufs=2, space="PSUM") as ppool,
            tc.tile_pool(name="psumt", bufs=2, space="PSUM") as tpool,
        ):
            wstat_sb = cpool.tile([128, 2048], BF16, tag="wstat")
            wih_sb = cpool.tile([128, 3072], BF16, tag="wih")
            idx_sb = cpool.tile([128, NTT], I32, tag="idx")
            mflag_sb = cpool.tile([128, NTT], BF16, tag="mflag")
            mbig_sb = cpool.tile([128, 8192], BF16, tag="mbig")
            xg = cpool.tile([128, NTT * EP], BF16, tag="xg")
            xt = cpool.tile([128, 3 * NTOK], BF16, tag="xt")
            zx = cpool.tile([128, 8 * NTOK], BF16, tag="zx")
            h_lo = cpool.tile([128, T * 64], BF16, tag="h_lo")
            h_hi = cpool.tile([128, T * 64], BF16, tag="h_hi")
            c_st = cpool.tile([128, 128], F32, tag="c_st")
            hzero = cpool.tile([128, 64], BF16, tag="hzero")
            sgif = cpool.tile([128, 256], BF16, tag="sgif")
            g_t = cpool.tile([128, 128], BF16, tag="g_t")
            o_t = cpool.tile([128, 128], BF16, tag="o_t")
            tmp_ig = cpool.tile([128, 128], BF16, tag="tmp_ig")
            tch = cpool.tile([128, 128], BF16, tag="tch")
            hm_t = cpool.tile([128, 128], BF16, tag="hm_t")
            hmax = cpool.tile([128, 128], F32, tag="hmax")
            ident = cpool.tile([128, 128], F32, tag="ident")
            ident_bf = cpool.tile([128, 128], BF16, tag="ident_bf")
            hmaxT = cpool.tile([128, 128], F32, tag="hmaxT")

            # recurrence PSUM: three bank-aligned (2KB) tiles, bufs=1
            zq_if = zpool.tile([128, 512], F32, tag="zq_if")
            zq_g = zpool.tile([128, 512], F32, tag="zq_g")
            zq_o = zpool.tile([128, 512], F32, tag="zq_o")

            nc.sync.dma_start(out=wstat_sb[:], in_=wstat_d[:, :])
            nc.sync.dma_start(out=wih_sb[:], in_=wih_d[:, :])
            nc.sync.dma_start(out=idx_sb[:], in_=idx_d[:, :])
            nc.sync.dma_start(out=mflag_sb[:], in_=mflag_d[:, :])
            nc.sync.dma_start(out=mbig_sb[:], in_=mbig_d[:, :])

            nc.vector.memset(c_st[:], 0.0)
            nc.vector.memset(hzero[:], 0.0)
            nc.vector.memset(hmax[:], BIGNEG)
            from concourse.masks import make_identity
            make_identity(nc, ident[:])
            nc.vector.tensor_copy(out=ident_bf[:], in_=ident[:])

            # ---------- Phase A emitters ----------
            def emit_group(grp):
                """Gather + flag lane + transpose for 4 tiles (512 tokens)."""
                items = []
                tk0 = grp * 4

                def gather():
                    for q in range(4):
                        tk = tk0 + q
                        nc.gpsimd.indirect_dma_start(
                            out=xg[:, tk * EP:(tk + 1) * EP],
                            out_offset=None,
                            in_=emb_d[:, :],
                            in_offset=bass.IndirectOffsetOnAxis(
                                ap=idx_sb[:, tk:tk + 1], axis=0),
                        )
                items.append(gather)

                def flags():
                    for q in range(4):
                        tk = tk0 + q
                        nc.vector.tensor_copy(
                            out=xg[:, tk * EP + 301:tk * EP + 302],
                            in_=mflag_sb[:, tk:tk + 1])
                items.append(flags)

                for kb in range(3):
                    def transp(kb=kb):
                        xtp = tpool.tile([128, 512], BF16, tag="xtp")
                        for q in range(4):
                            tk = tk0 + q
                            nc.tensor.transpose(
                                xtp[:, q * 128:(q + 1) * 128],
                                xg[:, tk * EP + kb * 128:
                                   tk * EP + (kb + 1) * 128],
                                ident_bf[:])
                        # xt col = kb*NTOK + s*64 + b  (s-major scan order)
                        if kb % 2 == 0:
                            nc.vector.tensor_copy(
                                out=xt[:, kb * NTOK + grp * 512:
                                       kb * NTOK + (grp + 1) * 512],
                                in_=xtp[:])
                        else:
                            nc.scalar.copy(
                                out=xt[:, kb * NTOK + grp * 512:
                                       kb * NTOK + (grp + 1) * 512],
                                in_=xtp[:])
                    items.append(transp)
                return items

            _nproj = [0]

            def emit_proj(ch, n):
                def proj():
                    zxp = ppool.tile([128, 512], F32, tag="zxp")
                    for kb in range(3):
                        nc.tensor.matmul(
                            zxp[:],
                            lhsT=wih_sb[:, (ch * 3 + kb) * 128:
                                        (ch * 3 + kb + 1) * 128],
                            rhs=xt[:, kb * NTOK + n * 512:
                                   kb * NTOK + (n + 1) * 512],
                            start=(kb == 0), stop=(kb == 2),
                        )
                    dst = zx[:, ch * 4096 + n * 512:ch * 4096 + (n + 1) * 512]
                    if _nproj[0] % 2 == 0:
                        nc.vector.tensor_copy(out=dst, in_=zxp[:])
                    else:
                        nc.scalar.copy(out=dst, in_=zxp[:])
                    _nproj[0] += 1
                return proj

            # pre-warm the PE during the gather window so projection and
            # the early recurrence run at K=8/8 (inputs land via DMA first)
            for _ in range(6):
                warm = ppool.tile([128, 512], F32, tag="zxp")
                nc.tensor.matmul(warm[:], lhsT=wstat_sb[:, 0:128],
                                 rhs=mbig_sb[:, 0:512],
                                 start=True, stop=True)
            # prologue: groups 0-1 gathered/transposed, projections n=0,1
            for it in emit_group(0):
                it()
            for ch in range(8):
                emit_proj(ch, 0)()
            for it in emit_group(1):
                it()
            for ch in range(8):
                emit_proj(ch, 1)()
            # deferred work queue: (group n, proj n) for n = 2..7, drained
            # at 2 items/step (large producer->consumer slack; tighter
            # just-in-time pacing exposed a missing-dep race)
            work = []
            for n in range(2, 8):
                work.extend(emit_group(n))
                for ch in range(8):
                    work.append(emit_proj(ch, n))

            # ---------- Phase B: recurrence ----------
            zx_v = zx[:].rearrange("p (c s b) -> p c s b", c=8, s=T)

            def pairs(zq, ch0, nch, s):
                for k in range(2):
                    for j in range(nch):
                        ch = ch0 + j
                        w_ap = wstat_sb[:, (ch * 2 + k) * 128:
                                        (ch * 2 + k + 1) * 128]
                        if s == 0:
                            rhs = hzero[:]
                        else:
                            hsrc = h_lo if k == 0 else h_hi
                            rhs = hsrc[:, (s - 1) * 64:s * 64]
                        nc.tensor.matmul(
                            zq[:, j * 64:(j + 1) * 64],
                            lhsT=w_ap, rhs=rhs,
                            start=False, stop=(k == 1),
                        )

            for s in range(T):
                # zx preloads (identity stationary; start clears the bank)
                nc.tensor.matmul(zq_if[:, 0:256], lhsT=ident_bf[:],
                                 rhs=zx_v[:, 0:4, s, :],
                                 start=True, stop=False)
                pairs(zq_if, 0, 4, s)
                nc.tensor.matmul(zq_g[:, 0:128], lhsT=ident_bf[:],
                                 rhs=zx_v[:, 4:6, s, :],
                                 start=True, stop=False)
                pairs(zq_g, 4, 2, s)
                nc.tensor.matmul(zq_o[:, 0:128], lhsT=ident_bf[:],
                                 rhs=zx_v[:, 6:8, s, :],
                                 start=True, stop=False)
                pairs(zq_o, 6, 2, s)

                nc.scalar.activation(sgif[:], zq_if[:, 0:256], AF.Sigmoid)
                nc.scalar.activation(g_t[:], zq_g[:, 0:128], AF.Tanh)
                nc.scalar.activation(o_t[:], zq_o[:, 0:128], AF.Sigmoid)
                nc.vector.tensor_mul(c_st[:], c_st[:], sgif[:, 128:256])
                nc.vector.tensor_mul(tmp_ig[:], sgif[:, 0:128], g_t[:])
                nc.vector.tensor_add(c_st[:], c_st[:], tmp_ig[:])
                nc.scalar.activation(tch[:, 0:64], c_st[:, 0:64], AF.Tanh)
                nc.scalar.activation(tch[:, 64:128], c_st[:, 64:128],
                                     AF.Tanh)
                h0 = h_lo[:, s * 64:(s + 1) * 64]
                h1 = h_hi[:, s * 64:(s + 1) * 64]
                nc.vector.tensor_mul(h0, o_t[:, 0:64], tch[:, 0:64])
                nc.vector.tensor_mul(h1, o_t[:, 64:128], tch[:, 64:128])
                # running masked max, off the h critical path
                nc.vector.tensor_add(hm_t[:, 0:64], h0,
                                     mbig_sb[:, s * 128:s * 128 + 64])
                nc.vector.tensor_add(hm_t[:, 64:128], h1,
                                     mbig_sb[:, s * 128 + 64:s * 128 + 128])
                nc.vector.tensor_max(hmax[:], hmax[:], hm_t[:])

                # stream phase A work into the PE's elementwise stall;
                # once drained, top up with dummy matmuls into the zxp
                # ring so HAM never re-throttles (no extra PSUM bank)
                if work:
                    for _ in range(2):
                        if work:
                            work.pop(0)()


            # ---------- Phase C: transpose + output ----------
            tp = ppool.tile([128, 512], F32, tag="zxp")
            nc.tensor.transpose(tp[:, 0:128], hmax[:], ident[:])
            nc.vector.tensor_copy(out=hmaxT[:], in_=tp[:, 0:128])
            # out[b, k*128 + p] <- hmaxT[j = k*64 + b, p]
            out_ap = bass.AP(tensor=out_d[:, :].tensor, offset=0,
                             ap=[[128, 2], [HID, NSC], [1, 128]])
            nc.sync.dma_start(out=out_ap, in_=hmaxT[:])

    nc.finalize()
    return nc


def _host_prep(token_ids, lengths, emb, w_ih_f, w_hh_f, b_f, w_ih_b, w_hh_b,
               b_b):
    emb384 = np.zeros((V, EP), dtype=bf)
    emb384[:, :E] = emb.astype(bf)
    emb384[:, 300] = bf(1.0)            # bias lane rides the gather

    wstat_d, wih_d = {}, {}
    for d in range(2):
        whh = w_hh_f if d == 0 else w_hh_b
        wstat = np.zeros((128, 2048), dtype=bf)
        for ch in range(8):
            gb = GB_BASE[ch]
            for k in range(2):
                blk = whh[gb:gb + 128, k * 128:(k + 1) * 128].T
                col = (ch * 2 + k) * 128
                wstat[:, col:col + 128] = blk.astype(bf)
        wstat_d[d] = wstat

        w_ih = w_ih_f if d == 0 else w_ih_b
        bias = b_f if d == 0 else b_b
        aug = np.zeros((EP, 4 * HID), dtype=np.float32)
        aug[:E, :] = w_ih.T
        aug[300, :] = bias
        if d == 1:
            mv = np.zeros(4 * HID, dtype=np.float32)
            mv[0:512] = BIGNEG          # i, f
            mv[768:1024] = BIGNEG       # o
            aug[301, :] = mv
        wih = np.zeros((128, 3072), dtype=bf)
        for ch in range(8):
            gb = GB_BASE[ch]
            for kb in range(3):
                blk = aug[kb * 128:(kb + 1) * 128, gb:gb + 128]
                col = (ch * 3 + kb) * 128
                wih[:, col:col + 128] = blk.astype(bf)
        wih_d[d] = wih

    in_maps = []
    for c in range(NCORES):
        d = 0 if c < 4 else 1
        blk = c % 4
        tok = token_ids[blk * NSC:(blk + 1) * NSC]      # [64, 64]
        ln = lengths[blk * NSC:(blk + 1) * NSC]         # [64]
        if d == 1:
            tok = tok[:, ::-1]                          # scan order = reversed

        # gather tile tk holds tokens (s = 2*tk + p//64, b = p%64)
        tok_sm = tok.T.reshape(NTT, 128)                # [s, b] -> tiles
        idx = tok_sm.T.astype(np.int32).copy()          # [128, NTT]

        ss = np.arange(T)[None, :]
        t_of_s = ss if d == 0 else T - 1 - ss
        pad = (t_of_s >= ln[:, None]).astype(np.float32)   # [b, s] by scan s
        mflag = pad.T.reshape(NTT, 128).T.astype(bf).copy()

        # mbig[p, s*128 + k*64 + b] = MAXNEG where padded (all p, both k)
        mb_ = np.zeros((T, 2, NSC), dtype=np.float32)
        mb_[:, :, :] = np.where(pad.T, MAXNEG, 0.0)[:, None, :]
        mb_ = np.broadcast_to(mb_.reshape(1, T * 128), (128, T * 128))
        in_maps.append({
            "emb": emb384,
            "idx": idx,
            "mflag": mflag,
            "wstat": wstat_d[d],
            "wih": wih_d[d],
            "mbig": mb_.astype(bf),
        })
    return in_maps


def kernel(token_ids, lengths, emb, w_ih_f, w_hh_f, b_f, w_ih_b, w_hh_b, b_b):
    global LAST_RESULTS
    if "nc" not in _CACHE:
        _CACHE["nc"] = _build_program()
    nc = _CACHE["nc"]
    in_maps = _host_prep(token_ids, lengths, emb, w_ih_f, w_hh_f, b_f,
                         w_ih_b, w_hh_b, b_b)
    res = bass_utils.run_bass_kernel_spmd(nc, in_maps, list(range(NCORES)))
    LAST_RESULTS = res
    out = np.zeros((B, 2 * HID), np.float32)
    for c in range(NCORES):
        d = 0 if c < 4 else 1
        blk = c % 4
        out[blk * NSC:(blk + 1) * NSC,
            d * HID:(d + 1) * HID] = res.results[c]["out"]
    return out


# revision 17
# speedup vs baseline: 1.0322x; 1.0322x over previous
"""Trainium2 Bass kernel for STSBaselineNet (embed -> biLSTM -> max-pool).

Sharding: one LSTM direction per core. Cores 0-3 run the forward pass of
sentence blocks 0-3; cores 4-7 run the backward pass of the same blocks
(time reversal and pad masking folded into host data prep: reversed token
order plus a -BIG pad-flag lane on the i/f/o logits).

v4 structure:
  - Token scan order is s-major so every PSUM->SBUF copy is contiguous.
  - The recurrence PSUM is split into three bank-aligned tiles
    [i,f | g | o], each preloaded with its zx slice by an identity
    -stationary matmul (start=True) that the W_hh pairs accumulate onto.
    Tile-granular dependency tracking then lets sigmoid(i,f) issue as
    soon as the i/f pairs stop, before the g/o matmuls finish.
  - Elementwise chain: sig(i,f) -> [c*=f | tmp=i*tanh(g)] -> c+=tmp ->
    tanh(c) -> h=o*tch, with the running masked max on GpSimd.
  - Phase A (gather / transpose / projection) is streamed into the
    recurrence as paced work items so the PE's elementwise stalls are
    filled with projection matmuls (keeps HAM at K=8/8).
"""

import numpy as np
import ml_dtypes

import concourse.bass as bass
import concourse.bacc as bacc
import concourse.mybir as mybir
import concourse.tile as tile
from concourse import bass_utils

V, E, HID, B, T = 50000, 300, 256, 256, 64
NCORES = 8
NSC = 64                    # sentences per core (one direction)
NTOK = NSC * T              # 4096 tokens/core
NTT = NTOK // 128           # 32 gather tiles
EP = 384                    # padded feature dim (300 emb + bias + flag + pad)
BIGNEG = -30.0              # logit offset for gate masking (bwd cores)
MAXNEG = -8.0               # mask offset for the final max (|h| < 1)

F32 = mybir.dt.float32
BF16 = mybir.dt.bfloat16
I32 = mybir.dt.int32
AF = mybir.ActivationFunctionType
OP = mybir.AluOpType

bf = ml_dtypes.bfloat16

# gate chunk order: [i i f f g g o o] == torch row order (256 rows each)
GB_BASE = {ch: ch * 128 for ch in range(8)}

_CACHE = {}
LAST_RESULTS = None


def _build_program():
    nc = bacc.Bacc(None, target_bir_lowering=False)

    emb_d = nc.dram_tensor("emb", [V, EP], BF16, kind="ExternalInput")
    idx_d = nc.dram_tensor("idx", [128, NTT], I32, kind="ExternalInput")
    mflag_d = nc.dram_tensor("mflag", [128, NTT], BF16, kind="ExternalInput")
    wstat_d = nc.dram_tensor("wstat", [128, 2048], BF16, kind="ExternalInput")
    wih_d = nc.dram_tensor("wih", [128, 3072], BF16, kind="ExternalInput")
    mbig_d = nc.dram_tensor("mbig", [128, 8192], BF16, kind="ExternalInput")
    out_d = nc.dram_tensor("out", [NSC, HID], F32, kind="ExternalOutput")

    with tile.TileContext(nc) as tc:
        with (
            tc.tile_pool(name="const", bufs=1) as cpool,
            tc.tile_pool(name="psumz", bufs=1, space="PSUM") as zpool,
            tc.tile_pool(name="psump", bufs=2, space="PSUM") as ppool,
            tc.tile_pool(name="psumt", bufs=2, space="PSUM") as tpool,
        ):
            wstat_sb = cpool.tile([128, 2048], BF16, tag="wstat")
            wih_sb = cpool.tile([128, 3072], BF16, tag="wih")
            idx_sb = cpool.tile([128, NTT], I32, tag="idx")
            mflag_sb = cpool.tile([128, NTT], BF16, tag="mflag")
            mbig_sb = cpool.tile([128, 8192], BF16, tag="mbig")
            xg = cpool.tile([128, NTT * EP], BF16, tag="xg")
            xt = cpool.tile([128, 3 * NTOK], BF16, tag="xt")
            zx = cpool.tile([128, 8 * NTOK], BF16, tag="zx")
            h_all = cpool.tile([128, T * 128], BF16, tag="h_all")
            c_st = cpool.tile([128, 128], F32, tag="c_st")
            hzero = cpool.tile([128, 64], BF16, tag="hzero")
            sgif = cpool.tile([128, 256], BF16, tag="sgif")
            g_t = cpool.tile([128, 128], BF16, tag="g_t")
            o_t = cpool.tile([128, 128], BF16, tag="o_t")
            tmp_ig = cpool.tile([128, 128], BF16, tag="tmp_ig")
            tch = cpool.tile([128, 128], BF16, tag="tch")
            hm_t = cpool.tile([128, 128], BF16, tag="hm_t")
            hmax = cpool.tile([128, 128], F32, tag="hmax")
            ident = cpool.tile([128, 128], F32, tag="ident")
            ident_bf = cpool.tile([128, 128], BF16, tag="ident_bf")
            hmaxT = cpool.tile([128, 128], F32, tag="hmaxT")

            # recurrence PSUM: three bank-aligned (2KB) tiles, bufs=1
            zq_if = zpool.tile([128, 512], F32, tag="zq_if")
            zq_g = zpool.tile([128, 512], F32, tag="zq_g")
            zq_o = zpool.tile([128, 512], F32, tag="zq_o")

            nc.sync.dma_start(out=wstat_sb[:], in_=wstat_d[:, :])
            nc.sync.dma_start(out=wih_sb[:], in_=wih_d[:, :])
            nc.sync.dma_start(out=idx_sb[:], in_=idx_d[:, :])
            nc.sync.dma_start(out=mflag_sb[:], in_=mflag_d[:, :])
            nc.sync.dma_start(out=mbig_sb[:], in_=mbig_d[:, :])

            nc.vector.memset(c_st[:], 0.0)
            nc.vector.memset(hzero[:], 0.0)
            nc.vector.memset(hmax[:], BIGNEG)
            from concourse.masks import make_identity
            make_identity(nc, ident[:])
            nc.vector.tensor_copy(out=ident_bf[:], in_=ident[:])

            # ---------- Phase A emitters ----------
            def emit_group(grp):
                """Gather + flag lane + transpose for 4 tiles (512 tokens)."""
                items = []
                tk0 = grp * 4

                def gather():
                    for q in range(4):
                        tk = tk0 + q
                        nc.gpsimd.indirect_dma_start(
                            out=xg[:, tk * EP:(tk + 1) * EP],
                            out_offset=None,
                            in_=emb_d[:, :],
                            in_offset=bass.IndirectOffsetOnAxis(
                                ap=idx_sb[:, tk:tk + 1], axis=0),
                        )
                items.append(gather)

                def flags():
                    for q in range(4):
                        tk = tk0 + q
                        nc.vector.tensor_copy(
                            out=xg[:, tk * EP + 301:tk * EP + 302],
                            in_=mflag_sb[:, tk:tk + 1])
                items.append(flags)

                for kb in range(3):
                    def transp(kb=kb):
                        xtp = tpool.tile([128, 512], BF16, tag="xtp")
                        for q in range(4):
                            tk = tk0 + q
                            nc.tensor.transpose(
                                xtp[:, q * 128:(q + 1) * 128],
                                xg[:, tk * EP + kb * 128:
                                   tk * EP + (kb + 1) * 128],
                                ident_bf[:])
                        # xt col = kb*NTOK + s*64 + b  (s-major scan order)
                        if kb % 2 == 0:
                            nc.vector.tensor_copy(
                                out=xt[:, kb * NTOK + grp * 512:
                                       kb * NTOK + (grp + 1) * 512],
                                in_=xtp[:])
                        else:
                            nc.scalar.copy(
                                out=xt[:, kb * NTOK + grp * 512:
                                       kb * NTOK + (grp + 1) * 512],
                                in_=xtp[:])
                    items.append(transp)
                return items

            _nproj = [0]

            def emit_proj(ch, n):
                def proj():
                    zxp = ppool.tile([128, 512], F32, tag="zxp")
                    for kb in range(3):
                        nc.tensor.matmul(
                            zxp[:],
                            lhsT=wih_sb[:, (ch * 3 + kb) * 128:
                                        (ch * 3 + kb + 1) * 128],
                            rhs=xt[:, kb * NTOK + n * 512:
                                   kb * NTOK + (n + 1) * 512],
                            start=(kb == 0), stop=(kb == 2),
                        )
                    dst = zx[:, ch * 4096 + n * 512:ch * 4096 + (n + 1) * 512]
                    if _nproj[0] % 2 == 0:
                        nc.vector.tensor_copy(out=dst, in_=zxp[:])
                    else:
                        nc.scalar.copy(out=dst, in_=zxp[:])
                    _nproj[0] += 1
                return proj

            # pre-warm the PE during the gather window so projection and
            # the early recurrence run at K=8/8 (inputs land via DMA first)
            for _ in range(6):
                warm = ppool.tile([128, 512], F32, tag="zxp")
                nc.tensor.matmul(warm[:], lhsT=wstat_sb[:, 0:128],
                                 rhs=mbig_sb[:, 0:512],
                                 start=True, stop=True)
            # prologue: groups 0-1 gathered/transposed, projections n=0,1
            for it in emit_group(0):
                it()
            for ch in range(8):
                emit_proj(ch, 0)()
            for it in emit_group(1):
                it()
            for ch in range(8):
                emit_proj(ch, 1)()
            # deferred work queue: (group n, proj n) for n = 2..7, drained
            # at 2 items/step (large producer->consumer slack; tighter
            # just-in-time pacing exposed a missing-dep race)
            work = []
            for n in range(2, 8):
                work.extend(emit_group(n))
                for ch in range(8):
                    work.append(emit_proj(ch, n))

            # ---------- Phase B: recurrence ----------
            zx_v = zx[:].rearrange("p (c s b) -> p c s b", c=8, s=T)

            def pairs(zq, ch0, nch, s):
                for j in range(nch):
                    ch = ch0 + j
                    for k in range(2):
                        w_ap = wstat_sb[:, (ch * 2 + k) * 128:
                                        (ch * 2 + k + 1) * 128]
                        if s == 0:
                            rhs = hzero[:]
                        else:
                            rhs = h_all[:, (s - 1) * 128 + k * 64:
                                        (s - 1) * 128 + (k + 1) * 64]
                        nc.tensor.matmul(
                            zq[:, j * 64:(j + 1) * 64],
                            lhsT=w_ap, rhs=rhs,
                            start=False, stop=(k == 1),
                        )

            for s in range(T):
                # zx preloads (identity stationary; start clears the bank)
                nc.tensor.matmul(zq_if[:, 0:256], lhsT=ident_bf[:],
                                 rhs=zx_v[:, 0:4, s, :],
                                 start=True, stop=False)
                pairs(zq_if, 0, 4, s)
                nc.tensor.matmul(zq_g[:, 0:128], lhsT=ident_bf[:],
                                 rhs=zx_v[:, 4:6, s, :],
                                 start=True, stop=False)
                pairs(zq_g, 4, 2, s)
                nc.tensor.matmul(zq_o[:, 0:128], lhsT=ident_bf[:],
                                 rhs=zx_v[:, 6:8, s, :],
                                 start=True, stop=False)
                pairs(zq_o, 6, 2, s)

                nc.scalar.activation(sgif[:], zq_if[:, 0:256], AF.Sigmoid)
                nc.scalar.activation(g_t[:], zq_g[:, 0:128], AF.Tanh)
                nc.scalar.activation(o_t[:], zq_o[:, 0:128], AF.Sigmoid)
                nc.vector.tensor_mul(c_st[:], c_st[:], sgif[:, 128:256])
                nc.vector.tensor_mul(tmp_ig[:], sgif[:, 0:128], g_t[:])
                nc.vector.tensor_add(c_st[:], c_st[:], tmp_ig[:])
                nc.scalar.activation(tch[:], c_st[:], AF.Tanh)
                hslot = h_all[:, s * 128:(s + 1) * 128]
                nc.vector.tensor_mul(hslot, o_t[:], tch[:])
                # running masked max, off the h critical path
                nc.vector.tensor_add(hm_t[:], hslot,
                                     mbig_sb[:, s * 128:(s + 1) * 128])
                nc.vector.tensor_max(hmax[:], hmax[:], hm_t[:])

                # stream phase A work into the PE's elementwise stall;
                # once drained, top up with dummy matmuls into the zxp
                # ring so HAM never re-throttles (no extra PSUM bank)
                if work:
                    for _ in range(2):
                        if work:
                            work.pop(0)()
                elif s < T - 2:
                    for _ in range(3):
                        warm = ppool.tile([128, 512], F32, tag="zxp")
                        nc.tensor.matmul(warm[:], lhsT=wih_sb[:, 0:128],
                                         rhs=xt[:, 0:512],
                                         start=True, stop=True)


            # ---------- Phase C: transpose + output ----------
            tp = ppool.tile([128, 512], F32, tag="zxp")
            nc.tensor.transpose(tp[:, 0:128], hmax[:], ident[:])
            nc.vector.tensor_copy(out=hmaxT[:], in_=tp[:, 0:128])
            # out[b, k*128 + p] <- hmaxT[j = k*64 + b, p]
            out_ap = bass.AP(tensor=out_d[:, :].tensor, offset=0,
                             ap=[[128, 2], [HID, NSC], [1, 128]])
            nc.sync.dma_start(out=out_ap, in_=hmaxT[:])

    nc.finalize()
    return nc


def _host_prep(token_ids, lengths, emb, w_ih_f, w_hh_f, b_f, w_ih_b, w_hh_b,
               b_b):
    emb384 = np.zeros((V, EP), dtype=bf)
    emb384[:, :E] = emb.astype(bf)
    emb384[:, 300] = bf(1.0)            # bias lane rides the gather

    wstat_d, wih_d = {}, {}
    for d in range(2):
        whh = w_hh_f if d == 0 else w_hh_b
        wstat = np.zeros((128, 2048), dtype=bf)
        for ch in range(8):
            gb = GB_BASE[ch]
            for k in range(2):
                blk = whh[gb:gb + 128, k * 128:(k + 1) * 128].T
                col = (ch * 2 + k) * 128
                wstat[:, col:col + 128] = blk.astype(bf)
        wstat_d[d] = wstat

        w_ih = w_ih_f if d == 0 else w_ih_b
        bias = b_f if d == 0 else b_b
        aug = np.zeros((EP, 4 * HID), dtype=np.float32)
        aug[:E, :] = w_ih.T
        aug[300, :] = bias
        if d == 1:
            mv = np.zeros(4 * HID, dtype=np.float32)
            mv[0:512] = BIGNEG          # i, f
            mv[768:1024] = BIGNEG       # o
            aug[301, :] = mv
        wih = np.zeros((128, 3072), dtype=bf)
        for ch in range(8):
            gb = GB_BASE[ch]
            for kb in range(3):
                blk = aug[kb * 128:(kb + 1) * 128, gb:gb + 128]
                col = (ch * 3 + kb) * 128
                wih[:, col:col + 128] = blk.astype(bf)
        wih_d[d] = wih

    in_maps = []
    for c in range(NCORES):
        d = 0 if c < 4 else 1
        blk = c % 4
        tok = token_ids[blk * NSC:(blk + 1) * NSC]      # [64, 64]
        ln = lengths[blk * NSC:(blk + 1) * NSC]         # [64]
        if d == 1:
            tok = tok[:, ::-1]                          # scan order = reversed

        # gather tile tk holds tokens (s = 2*tk + p//64, b = p%64)
        tok_sm = tok.T.reshape(NTT, 128)                # [s, b] -> tiles
        idx = tok_sm.T.astype(np.int32).copy()          # [128, NTT]

        ss = np.arange(T)[None, :]
        t_of_s = ss if d == 0 else T - 1 - ss
        pad = (t_of_s >= ln[:, None]).astype(np.float32)   # [b, s] by scan s
        mflag = pad.T.reshape(NTT, 128).T.astype(bf).copy()

        # mbig[p, s*128 + k*64 + b] = MAXNEG where padded (all p, both k)
        mb_ = np.zeros((T, 2, NSC), dtype=np.float32)
        mb_[:, :, :] = np.where(pad.T, MAXNEG, 0.0)[:, None, :]
        mb_ = np.broadcast_to(mb_.reshape(1, T * 128), (128, T * 128))
        in_maps.append({
            "emb": emb384,
            "idx": idx,
            "mflag": mflag,
            "wstat": wstat_d[d],
            "wih": wih_d[d],
            "mbig": mb_.astype(bf),
        })
    return in_maps


def kernel(token_ids, lengths, emb, w_ih_f, w_hh_f, b_f, w_ih_b, w_hh_b, b_b):
    global LAST_RESULTS
    if "nc" not in _CACHE:
        _CACHE["nc"] = _build_program()
    nc = _CACHE["nc"]
    in_maps = _host_prep(token_ids, lengths, emb, w_ih_f, w_hh_f, b_f,
                         w_ih_b, w_hh_b, b_b)
    res = bass_utils.run_bass_kernel_spmd(nc, in_maps, list(range(NCORES)))
    LAST_RESULTS = res
    out = np.zeros((B, 2 * HID), np.float32)
    for c in range(NCORES):
        d = 0 if c < 4 else 1
        blk = c % 4
        out[blk * NSC:(blk + 1) * NSC,
            d * HID:(d + 1) * HID] = res.results[c]["out"]
    return out


# revision 19
# speedup vs baseline: 1.0655x; 1.0322x over previous
"""Trainium2 Bass kernel for STSBaselineNet (embed -> biLSTM -> max-pool).

Sharding: one LSTM direction per core. Cores 0-3 run the forward pass of
sentence blocks 0-3; cores 4-7 run the backward pass of the same blocks
(time reversal and pad masking folded into host data prep: reversed token
order plus a -BIG pad-flag lane on the i/f/o logits).

Structure:
  - Token scan order is s-major so every PSUM->SBUF copy is contiguous.
  - The recurrence PSUM is split into three bank-aligned tiles
    [i,f | g | o], each preloaded with its zx slice by an identity
    -stationary matmul (start=True) that the W_hh pairs accumulate onto.
    Tile-granular dependency tracking then lets sigmoid(i,f) issue as
    soon as the i/f pairs stop, before the g/o matmuls finish.
  - Elementwise chain: sig(i,f) -> [c*=f | tmp=i*tanh(g)] -> c+=tmp ->
    tanh(c) -> h=o*tch, with the running masked max on the DVE.
  - Phase A (gather / transpose / projection) is streamed into the
    recurrence as paced work items so the PE's elementwise stalls are
    filled with projection matmuls (keeps HAM at K=8/8).
"""

import numpy as np
import ml_dtypes

import concourse.bass as bass
import concourse.bacc as bacc
import concourse.mybir as mybir
import concourse.tile as tile
from concourse import bass_utils

V, E, HID, B, T = 50000, 300, 256, 256, 64
NCORES = 8
NSC = 64                    # sentences per core (one direction)
NTOK = NSC * T              # 4096 tokens/core
NTT = NTOK // 128           # 32 gather tiles
EP = 384                    # padded feature dim (300 emb + bias + flag + pad)
BIGNEG = -30.0              # logit offset for gate masking (bwd cores)
MAXNEG = -8.0               # mask offset for the final max (|h| < 1)

F32 = mybir.dt.float32
BF16 = mybir.dt.bfloat16
I32 = mybir.dt.int32
AF = mybir.ActivationFunctionType
OP = mybir.AluOpType

bf = ml_dtypes.bfloat16

# gate chunk order: [i i f f g g o o] == torch row order (256 rows each)
GB_BASE = {ch: ch * 128 for ch in range(8)}

_CACHE = {}
LAST_RESULTS = None


def _build_program():
    nc = bacc.Bacc(None, target_bir_lowering=False)

    emb_d = nc.dram_tensor("emb", [V, EP], BF16, kind="ExternalInput")
    idx_d = nc.dram_tensor("idx", [128, NTT], I32, kind="ExternalInput")
    mflag_d = nc.dram_tensor("mflag", [128, NTT], BF16, kind="ExternalInput")
    wstat_d = nc.dram_tensor("wstat", [128, 2048], BF16, kind="ExternalInput")
    wih_d = nc.dram_tensor("wih", [128, 3072], BF16, kind="ExternalInput")
    mbig_d = nc.dram_tensor("mbig", [128, 8192], BF16, kind="ExternalInput")
    out_d = nc.dram_tensor("out", [NSC, HID], F32, kind="ExternalOutput")

    with tile.TileContext(nc) as tc:
        with (
            tc.tile_pool(name="const", bufs=1) as cpool,
            tc.tile_pool(name="psumz", bufs=1, space="PSUM") as zpool,
            tc.tile_pool(name="psump", bufs=2, space="PSUM") as ppool,
            tc.tile_pool(name="psumt", bufs=2, space="PSUM") as tpool,
        ):
            wstat_sb = cpool.tile([128, 2048], BF16, tag="wstat")
            wih_sb = cpool.tile([128, 3072], BF16, tag="wih")
            idx_sb = cpool.tile([128, NTT], I32, tag="idx")
            mflag_sb = cpool.tile([128, NTT], BF16, tag="mflag")
            mbig_sb = cpool.tile([128, 8192], BF16, tag="mbig")
            xg = cpool.tile([128, NTT * EP], BF16, tag="xg")
            xt = cpool.tile([128, 3 * NTOK], BF16, tag="xt")
            zx = cpool.tile([128, 8 * NTOK], BF16, tag="zx")
            h_all = cpool.tile([128, T * 128], BF16, tag="h_all")
            c_st = cpool.tile([128, 128], F32, tag="c_st")
            hzero = cpool.tile([128, 64], BF16, tag="hzero")
            sgif = cpool.tile([128, 256], BF16, tag="sgif")
            g_t = cpool.tile([128, 128], BF16, tag="g_t")
            o_t = cpool.tile([128, 128], BF16, tag="o_t")
            tmp_ig = cpool.tile([128, 128], BF16, tag="tmp_ig")
            tch = cpool.tile([128, 128], BF16, tag="tch")
            hm_t = cpool.tile([128, 128], BF16, tag="hm_t")
            hmax = cpool.tile([128, 128], F32, tag="hmax")
            ident = cpool.tile([128, 128], F32, tag="ident")
            ident_bf = cpool.tile([128, 128], BF16, tag="ident_bf")
            hmaxT = cpool.tile([128, 128], F32, tag="hmaxT")

            # recurrence PSUM: three bank-aligned (2KB) tiles, bufs=1
            zq_if = zpool.tile([128, 512], F32, tag="zq_if")
            zq_g = zpool.tile([128, 512], F32, tag="zq_g")
            zq_o = zpool.tile([128, 512], F32, tag="zq_o")

            nc.sync.dma_start(out=idx_sb[:], in_=idx_d[:, :])
            nc.sync.dma_start(out=mflag_sb[:], in_=mflag_d[:, :])
            nc.sync.dma_start(out=wstat_sb[:], in_=wstat_d[:, :])
            nc.sync.dma_start(out=wih_sb[:], in_=wih_d[:, :])
            nc.sync.dma_start(out=mbig_sb[:], in_=mbig_d[:, :])

            nc.vector.memset(c_st[:], 0.0)
            nc.vector.memset(hzero[:], 0.0)
            nc.vector.memset(hmax[:], BIGNEG)
            from concourse.masks import make_identity
            make_identity(nc, ident[:])
            nc.vector.tensor_copy(out=ident_bf[:], in_=ident[:])

            # ---------- Phase A emitters ----------
            def emit_group(grp):
                """Gather + flag lane + transpose for 4 tiles (512 tokens)."""
                items = []
                tk0 = grp * 4

                def gather():
                    for q in range(4):
                        tk = tk0 + q
                        nc.gpsimd.indirect_dma_start(
                            out=xg[:, tk * EP:(tk + 1) * EP],
                            out_offset=None,
                            in_=emb_d[:, :],
                            in_offset=bass.IndirectOffsetOnAxis(
                                ap=idx_sb[:, tk:tk + 1], axis=0),
                        )
                items.append(gather)

                def flags():
                    for q in range(4):
                        tk = tk0 + q
                        nc.vector.tensor_copy(
                            out=xg[:, tk * EP + 301:tk * EP + 302],
                            in_=mflag_sb[:, tk:tk + 1])
                items.append(flags)

                for kb in range(3):
                    def transp(kb=kb):
                        xtp = tpool.tile([128, 512], BF16, tag="xtp")
                        for q in range(4):
                            tk = tk0 + q
                            nc.tensor.transpose(
                                xtp[:, q * 128:(q + 1) * 128],
                                xg[:, tk * EP + kb * 128:
                                   tk * EP + (kb + 1) * 128],
                                ident_bf[:])
                        # xt col = kb*NTOK + s*64 + b  (s-major scan order)
                        if kb % 2 == 0:
                            nc.vector.tensor_copy(
                                out=xt[:, kb * NTOK + grp * 512:
                                       kb * NTOK + (grp + 1) * 512],
                                in_=xtp[:])
                        else:
                            nc.scalar.copy(
                                out=xt[:, kb * NTOK + grp * 512:
                                       kb * NTOK + (grp + 1) * 512],
                                in_=xtp[:])
                    items.append(transp)
                return items

            _nproj = [0]

            def emit_proj(ch, n):
                def proj():
                    zxp = ppool.tile([128, 512], F32, tag="zxp")
                    for kb in range(3):
                        nc.tensor.matmul(
                            zxp[:],
                            lhsT=wih_sb[:, (ch * 3 + kb) * 128:
                                        (ch * 3 + kb + 1) * 128],
                            rhs=xt[:, kb * NTOK + n * 512:
                                   kb * NTOK + (n + 1) * 512],
                            start=(kb == 0), stop=(kb == 2),
                        )
                    dst = zx[:, ch * 4096 + n * 512:ch * 4096 + (n + 1) * 512]
                    if _nproj[0] % 2 == 0:
                        nc.vector.tensor_copy(out=dst, in_=zxp[:])
                    else:
                        nc.scalar.copy(out=dst, in_=zxp[:])
                    _nproj[0] += 1
                return proj

            # pre-warm the PE during the gather window so projection and
            # the early recurrence run at K=8/8 (inputs land via DMA first)
            for _ in range(6):
                warm = ppool.tile([128, 512], F32, tag="zxp")
                nc.tensor.matmul(warm[:], lhsT=wstat_sb[:, 0:128],
                                 rhs=wstat_sb[:, 512:1024],
                                 start=True, stop=True)
            # prologue: groups 0-1 gathered/transposed, projections n=0,1
            for it in emit_group(0):
                it()
            for ch in range(8):
                emit_proj(ch, 0)()
            for it in emit_group(1):
                it()
            # deferred work queue: proj n=1 then (group n, proj n) for
            # n = 2..7, drained at 3 items/step (large producer->consumer
            # slack; tighter just-in-time pacing exposed a missing-dep race)
            work = []
            for ch in range(8):
                work.append(emit_proj(ch, 1))
            for n in range(2, 8):
                work.extend(emit_group(n))
                for ch in range(8):
                    work.append(emit_proj(ch, n))

            # ---------- Phase B: recurrence ----------
            zx_v = zx[:].rearrange("p (c s b) -> p c s b", c=8, s=T)

            def pairs(zq, ch0, nch, s):
                for j in range(nch):
                    ch = ch0 + j
                    for k in range(2):
                        w_ap = wstat_sb[:, (ch * 2 + k) * 128:
                                        (ch * 2 + k + 1) * 128]
                        if s == 0:
                            rhs = hzero[:]
                        else:
                            rhs = h_all[:, (s - 1) * 128 + k * 64:
                                        (s - 1) * 128 + (k + 1) * 64]
                        nc.tensor.matmul(
                            zq[:, j * 64:(j + 1) * 64],
                            lhsT=w_ap, rhs=rhs,
                            start=False, stop=(k == 1),
                        )

            for s in range(T):
                # zx preloads (identity stationary; start clears the bank)
                nc.tensor.matmul(zq_if[:, 0:256], lhsT=ident_bf[:],
                                 rhs=zx_v[:, 0:4, s, :],
                                 start=True, stop=False)
                pairs(zq_if, 0, 4, s)
                nc.tensor.matmul(zq_g[:, 0:128], lhsT=ident_bf[:],
                                 rhs=zx_v[:, 4:6, s, :],
                                 start=True, stop=False)
                pairs(zq_g, 4, 2, s)
                nc.tensor.matmul(zq_o[:, 0:128], lhsT=ident_bf[:],
                                 rhs=zx_v[:, 6:8, s, :],
                                 start=True, stop=False)
                pairs(zq_o, 6, 2, s)

                nc.scalar.activation(sgif[:], zq_if[:, 0:256], AF.Sigmoid)
                nc.scalar.activation(g_t[:], zq_g[:, 0:128], AF.Tanh)
                nc.scalar.activation(o_t[:], zq_o[:, 0:128], AF.Sigmoid)
                nc.vector.tensor_mul(c_st[:], c_st[:], sgif[:, 128:256])
                nc.vector.tensor_mul(tmp_ig[:], sgif[:, 0:128], g_t[:])
                nc.vector.tensor_add(c_st[:], c_st[:], tmp_ig[:])
                nc.scalar.activation(tch[:], c_st[:], AF.Tanh)
                hslot = h_all[:, s * 128:(s + 1) * 128]
                nc.vector.tensor_mul(hslot, o_t[:], tch[:])
                # running masked max, off the h critical path
                nc.vector.tensor_add(hm_t[:], hslot,
                                     mbig_sb[:, s * 128:(s + 1) * 128])
                nc.vector.tensor_max(hmax[:], hmax[:], hm_t[:])

                # stream phase A work into the PE's elementwise stall;
                # once drained, top up with dummy matmuls into the zxp
                # ring so HAM never re-throttles (no extra PSUM bank)
                if work:
                    for _ in range(3):
                        if work:
                            work.pop(0)()
                elif s < T - 2:
                    for _ in range(3):
                        warm = ppool.tile([128, 512], F32, tag="zxp")
                        nc.tensor.matmul(warm[:], lhsT=wih_sb[:, 0:128],
                                         rhs=xt[:, 0:512],
                                         start=True, stop=True)


            # ---------- Phase C: transpose + output ----------
            tp = ppool.tile([128, 512], F32, tag="zxp")
            nc.tensor.transpose(tp[:, 0:128], hmax[:], ident[:])
            nc.vector.tensor_copy(out=hmaxT[:], in_=tp[:, 0:128])
            # out[b, k*128 + p] <- hmaxT[j = k*64 + b, p]
            out_ap = bass.AP(tensor=out_d[:, :].tensor, offset=0,
                             ap=[[128, 2], [HID, NSC], [1, 128]])
            nc.sync.dma_start(out=out_ap, in_=hmaxT[:])

    nc.finalize()
    return nc


def _host_prep(token_ids, lengths, emb, w_ih_f, w_hh_f, b_f, w_ih_b, w_hh_b,
               b_b):
    emb384 = np.zeros((V, EP), dtype=bf)
    emb384[:, :E] = emb.astype(bf)
    emb384[:, 300] = bf(1.0)            # bias lane rides the gather

    wstat_d, wih_d = {}, {}
    for d in range(2):
        whh = w_hh_f if d == 0 else w_hh_b
        wstat = np.zeros((128, 2048), dtype=bf)
        for ch in range(8):
            gb = GB_BASE[ch]
            for k in range(2):
                blk = whh[gb:gb + 128, k * 128:(k + 1) * 128].T
                col = (ch * 2 + k) * 128
                wstat[:, col:col + 128] = blk.astype(bf)
        wstat_d[d] = wstat

        w_ih = w_ih_f if d == 0 else w_ih_b
        bias = b_f if d == 0 else b_b
        aug = np.zeros((EP, 4 * HID), dtype=np.float32)
        aug[:E, :] = w_ih.T
        aug[300, :] = bias
        if d == 1:
            mv = np.zeros(4 * HID, dtype=np.float32)
            mv[0:512] = BIGNEG          # i, f
            mv[768:1024] = BIGNEG       # o
            aug[301, :] = mv
        wih = np.zeros((128, 3072), dtype=bf)
        for ch in range(8):
            gb = GB_BASE[ch]
            for kb in range(3):
                blk = aug[kb * 128:(kb + 1) * 128, gb:gb + 128]
                col = (ch * 3 + kb) * 128
                wih[:, col:col + 128] = blk.astype(bf)
        wih_d[d] = wih

    in_maps = []
    for c in range(NCORES):
        d = 0 if c < 4 else 1
        blk = c % 4
        tok = token_ids[blk * NSC:(blk + 1) * NSC]      # [64, 64]
        ln = lengths[blk * NSC:(blk + 1) * NSC]         # [64]
        if d == 1:
            tok = tok[:, ::-1]                          # scan order = reversed

        # gather tile tk holds tokens (s = 2*tk + p//64, b = p%64)
        tok_sm = tok.T.reshape(NTT, 128)                # [s, b] -> tiles
        idx = tok_sm.T.astype(np.int32).copy()          # [128, NTT]

        ss = np.arange(T)[None, :]
        t_of_s = ss if d == 0 else T - 1 - ss
        pad = (t_of_s >= ln[:, None]).astype(np.float32)   # [b, s] by scan s
        mflag = pad.T.reshape(NTT, 128).T.astype(bf).copy()

        # mbig[p, s*128 + k*64 + b] = MAXNEG where padded (all p, both k)
        mb_ = np.zeros((T, 2, NSC), dtype=np.float32)
        mb_[:, :, :] = np.where(pad.T, MAXNEG, 0.0)[:, None, :]
        mb_ = np.broadcast_to(mb_.reshape(1, T * 128), (128, T * 128))
        in_maps.append({
            "emb": emb384,
            "idx": idx,
            "mflag": mflag,
            "wstat": wstat_d[d],
            "wih": wih_d[d],
            "mbig": mb_.astype(bf),
        })
    return in_maps


def kernel(token_ids, lengths, emb, w_ih_f, w_hh_f, b_f, w_ih_b, w_hh_b, b_b):
    global LAST_RESULTS
    if "nc" not in _CACHE:
        _CACHE["nc"] = _build_program()
    nc = _CACHE["nc"]
    in_maps = _host_prep(token_ids, lengths, emb, w_ih_f, w_hh_f, b_f,
                         w_ih_b, w_hh_b, b_b)
    res = bass_utils.run_bass_kernel_spmd(nc, in_maps, list(range(NCORES)))
    LAST_RESULTS = res
    out = np.zeros((B, 2 * HID), np.float32)
    for c in range(NCORES):
        d = 0 if c < 4 else 1
        blk = c % 4
        out[blk * NSC:(blk + 1) * NSC,
            d * HID:(d + 1) * HID] = res.results[c]["out"]
    return out


# revision 20
# speedup vs baseline: 1.0656x; 1.0001x over previous
"""Trainium2 Bass kernel for STSBaselineNet (embed -> biLSTM -> max-pool).

Sharding: one LSTM direction per core. Cores 0-3 run the forward pass of
sentence blocks 0-3; cores 4-7 run the backward pass of the same blocks
(time reversal and pad masking folded into host data prep: reversed token
order plus a -BIG pad-flag lane on the i/f/o logits).

Structure:
  - Token scan order is s-major so every PSUM->SBUF copy is contiguous.
  - The recurrence PSUM is split into three bank-aligned tiles
    [i,f | g | o], each preloaded with its zx slice by an identity
    -stationary matmul (start=True) that the W_hh pairs accumulate onto.
    Tile-granular dependency tracking then lets sigmoid(i,f) issue as
    soon as the i/f pairs stop, before the g/o matmuls finish.
  - Elementwise chain: sig(i,f) -> [c*=f | tmp=i*tanh(g)] -> c+=tmp ->
    tanh(c) -> h=o*tch, with the running masked max on the DVE.
  - Phase A (gather / transpose / projection) is streamed into the
    recurrence as paced work items so the PE's elementwise stalls are
    filled with projection matmuls (keeps HAM at K=8/8).
"""

import numpy as np
import ml_dtypes

import concourse.bass as bass
import concourse.bacc as bacc
import concourse.mybir as mybir
import concourse.tile as tile
from concourse import bass_utils

V, E, HID, B, T = 50000, 300, 256, 256, 64
NCORES = 8
NSC = 64                    # sentences per core (one direction)
NTOK = NSC * T              # 4096 tokens/core
NTT = NTOK // 128           # 32 gather tiles
EP = 384                    # padded feature dim (300 emb + bias + flag + pad)
BIGNEG = -30.0              # logit offset for gate masking (bwd cores)
MAXNEG = -8.0               # mask offset for the final max (|h| < 1)

F32 = mybir.dt.float32
BF16 = mybir.dt.bfloat16
I32 = mybir.dt.int32
AF = mybir.ActivationFunctionType
OP = mybir.AluOpType

bf = ml_dtypes.bfloat16

# gate chunk order: [i i f f g g o o] == torch row order (256 rows each)
GB_BASE = {ch: ch * 128 for ch in range(8)}

_CACHE = {}
LAST_RESULTS = None


def _build_program():
    nc = bacc.Bacc(None, target_bir_lowering=False)

    emb_d = nc.dram_tensor("emb", [V, EP], BF16, kind="ExternalInput")
    idx_d = nc.dram_tensor("idx", [128, NTT], I32, kind="ExternalInput")
    mflag_d = nc.dram_tensor("mflag", [128, NTT], BF16, kind="ExternalInput")
    wstat_d = nc.dram_tensor("wstat", [128, 2048], BF16, kind="ExternalInput")
    wih_d = nc.dram_tensor("wih", [128, 3072], BF16, kind="ExternalInput")
    mbig_d = nc.dram_tensor("mbig", [128, 8192], BF16, kind="ExternalInput")
    out_d = nc.dram_tensor("out", [NSC, HID], F32, kind="ExternalOutput")

    with tile.TileContext(nc) as tc:
        with (
            tc.tile_pool(name="const", bufs=1) as cpool,
            tc.tile_pool(name="psumz", bufs=1, space="PSUM") as zpool,
            tc.tile_pool(name="psump", bufs=2, space="PSUM") as ppool,
            tc.tile_pool(name="psumt", bufs=2, space="PSUM") as tpool,
        ):
            wstat_sb = cpool.tile([128, 2048], BF16, tag="wstat")
            wih_sb = cpool.tile([128, 3072], BF16, tag="wih")
            idx_sb = cpool.tile([128, NTT], I32, tag="idx")
            mflag_sb = cpool.tile([128, NTT], BF16, tag="mflag")
            mbig_sb = cpool.tile([128, 8192], BF16, tag="mbig")
            xg = cpool.tile([128, NTT * EP], BF16, tag="xg")
            xt = cpool.tile([128, 3 * NTOK], BF16, tag="xt")
            zx = cpool.tile([128, 8 * NTOK], BF16, tag="zx")
            h_all = cpool.tile([128, T * 128], BF16, tag="h_all")
            c_st = cpool.tile([128, 128], F32, tag="c_st")
            hzero = cpool.tile([128, 64], BF16, tag="hzero")
            sgif = cpool.tile([128, 256], BF16, tag="sgif")
            g_t = cpool.tile([128, 128], BF16, tag="g_t")
            o_t = cpool.tile([128, 128], BF16, tag="o_t")
            tmp_ig = cpool.tile([128, 128], BF16, tag="tmp_ig")
            tch = cpool.tile([128, 128], BF16, tag="tch")
            hm_t = cpool.tile([128, 128], BF16, tag="hm_t")
            hmax = cpool.tile([128, 128], F32, tag="hmax")
            ident = cpool.tile([128, 128], F32, tag="ident")
            ident_bf = cpool.tile([128, 128], BF16, tag="ident_bf")
            hmaxT = cpool.tile([128, 128], F32, tag="hmaxT")

            # recurrence PSUM: three bank-aligned (2KB) tiles, bufs=1
            zq_if = zpool.tile([128, 512], F32, tag="zq_if")
            zq_g = zpool.tile([128, 512], F32, tag="zq_g")
            zq_o = zpool.tile([128, 512], F32, tag="zq_o")

            nc.sync.dma_start(out=idx_sb[:], in_=idx_d[:, :])
            nc.sync.dma_start(out=mflag_sb[:], in_=mflag_d[:, :])
            nc.sync.dma_start(out=wstat_sb[:], in_=wstat_d[:, :])
            nc.sync.dma_start(out=wih_sb[:], in_=wih_d[:, :])
            nc.sync.dma_start(out=mbig_sb[:], in_=mbig_d[:, :])

            nc.vector.memset(c_st[:], 0.0)
            nc.vector.memset(hzero[:], 0.0)
            nc.vector.memset(hmax[:], BIGNEG)
            from concourse.masks import make_identity
            make_identity(nc, ident[:])
            nc.vector.tensor_copy(out=ident_bf[:], in_=ident[:])

            # ---------- Phase A emitters ----------
            def emit_group(grp):
                """Gather + flag lane + transpose for 4 tiles (512 tokens)."""
                items = []
                tk0 = grp * 4

                def gather():
                    for q in range(4):
                        tk = tk0 + q
                        nc.gpsimd.indirect_dma_start(
                            out=xg[:, tk * EP:(tk + 1) * EP],
                            out_offset=None,
                            in_=emb_d[:, :],
                            in_offset=bass.IndirectOffsetOnAxis(
                                ap=idx_sb[:, tk:tk + 1], axis=0),
                        )
                items.append(gather)

                def flags():
                    for q in range(4):
                        tk = tk0 + q
                        nc.vector.tensor_copy(
                            out=xg[:, tk * EP + 301:tk * EP + 302],
                            in_=mflag_sb[:, tk:tk + 1])
                items.append(flags)

                for kb in range(3):
                    def transp(kb=kb):
                        xtp = tpool.tile([128, 512], BF16, tag="xtp")
                        for q in range(4):
                            tk = tk0 + q
                            nc.tensor.transpose(
                                xtp[:, q * 128:(q + 1) * 128],
                                xg[:, tk * EP + kb * 128:
                                   tk * EP + (kb + 1) * 128],
                                ident_bf[:])
                        # xt col = kb*NTOK + s*64 + b  (s-major scan order)
                        if kb % 2 == 0:
                            nc.vector.tensor_copy(
                                out=xt[:, kb * NTOK + grp * 512:
                                       kb * NTOK + (grp + 1) * 512],
                                in_=xtp[:])
                        else:
                            nc.scalar.copy(
                                out=xt[:, kb * NTOK + grp * 512:
                                       kb * NTOK + (grp + 1) * 512],
                                in_=xtp[:])
                    items.append(transp)
                return items

            _nproj = [0]

            def emit_proj(ch, n):
                def proj():
                    zxp = ppool.tile([128, 512], F32, tag="zxp")
                    for kb in range(3):
                        nc.tensor.matmul(
                            zxp[:],
                            lhsT=wih_sb[:, (ch * 3 + kb) * 128:
                                        (ch * 3 + kb + 1) * 128],
                            rhs=xt[:, kb * NTOK + n * 512:
                                   kb * NTOK + (n + 1) * 512],
                            start=(kb == 0), stop=(kb == 2),
                        )
                    dst = zx[:, ch * 4096 + n * 512:ch * 4096 + (n + 1) * 512]
                    if _nproj[0] % 2 == 0:
                        nc.vector.tensor_copy(out=dst, in_=zxp[:])
                    else:
                        nc.scalar.copy(out=dst, in_=zxp[:])
                    _nproj[0] += 1
                return proj

            # pre-warm the PE during the gather window so projection and
            # the early recurrence run at K=8/8 (inputs land via DMA first)
            for _ in range(6):
                warm = ppool.tile([128, 512], F32, tag="zxp")
                nc.tensor.matmul(warm[:], lhsT=wstat_sb[:, 0:128],
                                 rhs=wstat_sb[:, 512:1024],
                                 start=True, stop=True)
            # prologue: groups 0-1 gathered/transposed, projections n=0,1
            for it in emit_group(0):
                it()
            for ch in range(8):
                emit_proj(ch, 0)()
            for it in emit_group(1):
                it()
            # deferred work queue: proj n=1 then (group n, proj n) for
            # n = 2..7, drained at 3 items/step (large producer->consumer
            # slack; tighter just-in-time pacing exposed a missing-dep race)
            work = []
            for ch in range(8):
                work.append(emit_proj(ch, 1))
            for n in range(2, 8):
                work.extend(emit_group(n))
                for ch in range(8):
                    work.append(emit_proj(ch, n))

            # ---------- Phase B: recurrence ----------
            zx_v = zx[:].rearrange("p (c s b) -> p c s b", c=8, s=T)

            def pairs(zq, ch0, nch, s):
                for j in range(nch):
                    ch = ch0 + j
                    for k in range(2):
                        w_ap = wstat_sb[:, (ch * 2 + k) * 128:
                                        (ch * 2 + k + 1) * 128]
                        if s == 0:
                            rhs = hzero[:]
                        else:
                            rhs = h_all[:, (s - 1) * 128 + k * 64:
                                        (s - 1) * 128 + (k + 1) * 64]
                        nc.tensor.matmul(
                            zq[:, j * 64:(j + 1) * 64],
                            lhsT=w_ap, rhs=rhs,
                            start=False, stop=(k == 1),
                        )

            for s in range(T):
                # zx preloads (identity stationary; start clears the bank)
                nc.tensor.matmul(zq_if[:, 0:256], lhsT=ident_bf[:],
                                 rhs=zx_v[:, 0:4, s, :],
                                 start=True, stop=False)
                pairs(zq_if, 0, 4, s)
                nc.tensor.matmul(zq_g[:, 0:128], lhsT=ident_bf[:],
                                 rhs=zx_v[:, 4:6, s, :],
                                 start=True, stop=False)
                pairs(zq_g, 4, 2, s)
                nc.tensor.matmul(zq_o[:, 0:128], lhsT=ident_bf[:],
                                 rhs=zx_v[:, 6:8, s, :],
                                 start=True, stop=False)
                pairs(zq_o, 6, 2, s)

                nc.scalar.activation(sgif[:], zq_if[:, 0:256], AF.Sigmoid)
                nc.scalar.activation(g_t[:], zq_g[:, 0:128], AF.Tanh)
                nc.scalar.activation(o_t[:], zq_o[:, 0:128], AF.Sigmoid)
                nc.vector.tensor_mul(c_st[:], c_st[:], sgif[:, 128:256])
                nc.vector.tensor_mul(tmp_ig[:], sgif[:, 0:128], g_t[:])
                nc.vector.tensor_add(c_st[:], c_st[:], tmp_ig[:])
                nc.scalar.activation(tch[:], c_st[:], AF.Tanh)
                hslot = h_all[:, s * 128:(s + 1) * 128]
                nc.vector.tensor_mul(hslot, o_t[:], tch[:])
                # running masked max, off the h critical path
                nc.vector.tensor_add(hm_t[:], hslot,
                                     mbig_sb[:, s * 128:(s + 1) * 128])
                nc.vector.tensor_max(hmax[:], hmax[:], hm_t[:])

                # stream phase A work into the PE's elementwise stall;
                # once drained, top up with dummy matmuls into the zxp
                # ring so HAM never re-throttles (no extra PSUM bank)
                if work and s >= 2:
                    for _ in range(3):
                        if work:
                            work.pop(0)()
                elif s < T - 2:
                    for _ in range(3):
                        warm = ppool.tile([128, 512], F32, tag="zxp")
                        nc.tensor.matmul(warm[:], lhsT=wih_sb[:, 0:128],
                                         rhs=xt[:, 0:512],
                                         start=True, stop=True)


            # ---------- Phase C: transpose + output ----------
            tp = ppool.tile([128, 512], F32, tag="zxp")
            nc.tensor.transpose(tp[:, 0:128], hmax[:], ident[:])
            nc.vector.tensor_copy(out=hmaxT[:], in_=tp[:, 0:128])
            # out[b, k*128 + p] <- hmaxT[j = k*64 + b, p]
            out_ap = bass.AP(tensor=out_d[:, :].tensor, offset=0,
                             ap=[[128, 2], [HID, NSC], [1, 128]])
            nc.sync.dma_start(out=out_ap, in_=hmaxT[:])

    nc.finalize()
    return nc


def _host_prep(token_ids, lengths, emb, w_ih_f, w_hh_f, b_f, w_ih_b, w_hh_b,
               b_b):
    emb384 = np.zeros((V, EP), dtype=bf)
    emb384[:, :E] = emb.astype(bf)
    emb384[:, 300] = bf(1.0)            # bias lane rides the gather

    wstat_d, wih_d = {}, {}
    for d in range(2):
        whh = w_hh_f if d == 0 else w_hh_b
        wstat = np.zeros((128, 2048), dtype=bf)
        for ch in range(8):
            gb = GB_BASE[ch]
            for k in range(2):
                blk = whh[gb:gb + 128, k * 128:(k + 1) * 128].T
                col = (ch * 2 + k) * 128
                wstat[:, col:col + 128] = blk.astype(bf)
        wstat_d[d] = wstat

        w_ih = w_ih_f if d == 0 else w_ih_b
        bias = b_f if d == 0 else b_b
        aug = np.zeros((EP, 4 * HID), dtype=np.float32)
        aug[:E, :] = w_ih.T
        aug[300, :] = bias
        if d == 1:
            mv = np.zeros(4 * HID, dtype=np.float32)
            mv[0:512] = BIGNEG          # i, f
            mv[768:1024] = BIGNEG       # o
            aug[301, :] = mv
        wih = np.zeros((128, 3072), dtype=bf)
        for ch in range(8):
            gb = GB_BASE[ch]
            for kb in range(3):
                blk = aug[kb * 128:(kb + 1) * 128, gb:gb + 128]
                col = (ch * 3 + kb) * 128
                wih[:, col:col + 128] = blk.astype(bf)
        wih_d[d] = wih

    in_maps = []
    for c in range(NCORES):
        d = 0 if c < 4 else 1
        blk = c % 4
        tok = token_ids[blk * NSC:(blk + 1) * NSC]      # [64, 64]
        ln = lengths[blk * NSC:(blk + 1) * NSC]         # [64]
        if d == 1:
            tok = tok[:, ::-1]                          # scan order = reversed

        # gather tile tk holds tokens (s = 2*tk + p//64, b = p%64)
        tok_sm = tok.T.reshape(NTT, 128)                # [s, b] -> tiles
        idx = tok_sm.T.astype(np.int32).copy()          # [128, NTT]

        ss = np.arange(T)[None, :]
        t_of_s = ss if d == 0 else T - 1 - ss
        pad = (t_of_s >= ln[:, None]).astype(np.float32)   # [b, s] by scan s
        mflag = pad.T.reshape(NTT, 128).T.astype(bf).copy()

        # mbig[p, s*128 + k*64 + b] = MAXNEG where padded (all p, both k)
        mb_ = np.zeros((T, 2, NSC), dtype=np.float32)
        mb_[:, :, :] = np.where(pad.T, MAXNEG, 0.0)[:, None, :]
        mb_ = np.broadcast_to(mb_.reshape(1, T * 128), (128, T * 128))
        in_maps.append({
            "emb": emb384,
            "idx": idx,
            "mflag": mflag,
            "wstat": wstat_d[d],
            "wih": wih_d[d],
            "mbig": mb_.astype(bf),
        })
    return in_maps


def kernel(token_ids, lengths, emb, w_ih_f, w_hh_f, b_f, w_ih_b, w_hh_b, b_b):
    global LAST_RESULTS
    if "nc" not in _CACHE:
        _CACHE["nc"] = _build_program()
    nc = _CACHE["nc"]
    in_maps = _host_prep(token_ids, lengths, emb, w_ih_f, w_hh_f, b_f,
                         w_ih_b, w_hh_b, b_b)
    res = bass_utils.run_bass_kernel_spmd(nc, in_maps, list(range(NCORES)))
    LAST_RESULTS = res
    out = np.zeros((B, 2 * HID), np.float32)
    for c in range(NCORES):
        d = 0 if c < 4 else 1
        blk = c % 4
        out[blk * NSC:(blk + 1) * NSC,
            d * HID:(d + 1) * HID] = res.results[c]["out"]
    return out


# revision 22
# speedup vs baseline: 1.0678x; 1.0021x over previous
"""Trainium2 Bass kernel for STSBaselineNet (embed -> biLSTM -> max-pool).

Sharding: one LSTM direction per core. Cores 0-3 run the forward pass of
sentence blocks 0-3; cores 4-7 run the backward pass of the same blocks
(time reversal and pad masking folded into host data prep: reversed token
order plus a -BIG pad-flag lane on the i/f/o logits).

Structure:
  - Token scan order is s-major so every PSUM->SBUF copy is contiguous.
  - The recurrence PSUM is split into three bank-aligned tiles
    [i,f | g | o], each preloaded with its zx slice by an identity
    -stationary matmul (start=True) that the W_hh pairs accumulate onto.
    Tile-granular dependency tracking then lets sigmoid(i,f) issue as
    soon as the i/f pairs stop, before the g/o matmuls finish.
  - Elementwise chain: sig(i,f) -> [c*=f | tmp=i*tanh(g)] -> c+=tmp ->
    tanh(c) -> h=o*tch, with the running masked max on the DVE.
  - Phase A (gather / transpose / projection) is streamed into the
    recurrence as paced work items so the PE's elementwise stalls are
    filled with projection matmuls (keeps HAM at K=8/8).
"""

import numpy as np
import ml_dtypes

import concourse.bass as bass
import concourse.bacc as bacc
import concourse.mybir as mybir
import concourse.tile as tile
from concourse import bass_utils

V, E, HID, B, T = 50000, 300, 256, 256, 64
NCORES = 8
NSC = 64                    # sentences per core (one direction)
NTOK = NSC * T              # 4096 tokens/core
NTT = NTOK // 128           # 32 gather tiles
EP = 384                    # padded feature dim (300 emb + bias + flag + pad)
BIGNEG = -30.0              # logit offset for gate masking (bwd cores)
MAXNEG = -8.0               # mask offset for the final max (|h| < 1)

F32 = mybir.dt.float32
BF16 = mybir.dt.bfloat16
I32 = mybir.dt.int32
AF = mybir.ActivationFunctionType
OP = mybir.AluOpType

bf = ml_dtypes.bfloat16

# gate chunk order: [i i f f g g o o] == torch row order (256 rows each)
GB_BASE = {ch: ch * 128 for ch in range(8)}

_CACHE = {}
LAST_RESULTS = None


def _build_program():
    nc = bacc.Bacc(None, target_bir_lowering=False)

    emb_d = nc.dram_tensor("emb", [V, EP], BF16, kind="ExternalInput")
    idx_d = nc.dram_tensor("idx", [128, NTT], I32, kind="ExternalInput")
    mflag_d = nc.dram_tensor("mflag", [128, NTT], BF16, kind="ExternalInput")
    wstat_d = nc.dram_tensor("wstat", [128, 2048], BF16, kind="ExternalInput")
    wih_d = nc.dram_tensor("wih", [128, 3072], BF16, kind="ExternalInput")
    mbig_d = nc.dram_tensor("mbig", [128, 8192], BF16, kind="ExternalInput")
    out_d = nc.dram_tensor("out", [NSC, HID], F32, kind="ExternalOutput")

    with tile.TileContext(nc) as tc:
        with (
            tc.tile_pool(name="const", bufs=1) as cpool,
            tc.tile_pool(name="psumz", bufs=1, space="PSUM") as zpool,
            tc.tile_pool(name="psump", bufs=2, space="PSUM") as ppool,
            tc.tile_pool(name="psumt", bufs=2, space="PSUM") as tpool,
        ):
            wstat_sb = cpool.tile([128, 2048], BF16, tag="wstat")
            wih_sb = cpool.tile([128, 3072], BF16, tag="wih")
            idx_sb = cpool.tile([128, NTT], I32, tag="idx")
            mflag_sb = cpool.tile([128, NTT], BF16, tag="mflag")
            mbig_sb = cpool.tile([128, 8192], BF16, tag="mbig")
            xg = cpool.tile([128, NTT * EP], BF16, tag="xg")
            xt = cpool.tile([128, 3 * NTOK], BF16, tag="xt")
            zx = cpool.tile([128, 8 * NTOK], BF16, tag="zx")
            h_all = cpool.tile([128, T * 128], BF16, tag="h_all")
            c_st = cpool.tile([128, 128], F32, tag="c_st")
            hzero = cpool.tile([128, 64], BF16, tag="hzero")
            sgif = cpool.tile([128, 256], BF16, tag="sgif")
            g_t = cpool.tile([128, 128], BF16, tag="g_t")
            o_t = cpool.tile([128, 128], BF16, tag="o_t")
            tmp_ig = cpool.tile([128, 128], BF16, tag="tmp_ig")
            tch = cpool.tile([128, 128], BF16, tag="tch")
            hm_t = cpool.tile([128, 128], BF16, tag="hm_t")
            hmax = cpool.tile([128, 128], F32, tag="hmax")
            ident = cpool.tile([128, 128], F32, tag="ident")
            ident_bf = cpool.tile([128, 128], BF16, tag="ident_bf")
            hmaxT = cpool.tile([128, 128], F32, tag="hmaxT")

            # recurrence PSUM: three bank-aligned (2KB) tiles, bufs=1
            zq_if = zpool.tile([128, 512], F32, tag="zq_if")
            zq_g = zpool.tile([128, 512], F32, tag="zq_g")
            zq_o = zpool.tile([128, 512], F32, tag="zq_o")

            nc.sync.dma_start(out=idx_sb[:], in_=idx_d[:, :])
            nc.sync.dma_start(out=mflag_sb[:], in_=mflag_d[:, :])
            nc.sync.dma_start(out=wstat_sb[:], in_=wstat_d[:, :])
            nc.sync.dma_start(out=wih_sb[:], in_=wih_d[:, :])
            nc.sync.dma_start(out=mbig_sb[:], in_=mbig_d[:, :])

            nc.vector.memset(c_st[:], 0.0)
            nc.vector.memset(hzero[:], 0.0)
            nc.vector.memset(hmax[:], BIGNEG)
            from concourse.masks import make_identity
            make_identity(nc, ident[:])
            nc.vector.tensor_copy(out=ident_bf[:], in_=ident[:])

            # ---------- Phase A emitters ----------
            def emit_group(grp):
                """Gather + flag lane + transpose for 4 tiles (512 tokens)."""
                items = []
                tk0 = grp * 4

                def gather():
                    for q in range(4):
                        tk = tk0 + q
                        nc.gpsimd.indirect_dma_start(
                            out=xg[:, tk * EP:(tk + 1) * EP],
                            out_offset=None,
                            in_=emb_d[:, :],
                            in_offset=bass.IndirectOffsetOnAxis(
                                ap=idx_sb[:, tk:tk + 1], axis=0),
                        )
                items.append(gather)

                def flags():
                    for q in range(4):
                        tk = tk0 + q
                        nc.vector.tensor_copy(
                            out=xg[:, tk * EP + 301:tk * EP + 302],
                            in_=mflag_sb[:, tk:tk + 1])
                items.append(flags)

                for kb in range(3):
                    def transp(kb=kb):
                        xtp = tpool.tile([128, 512], BF16, tag="xtp")
                        for q in range(4):
                            tk = tk0 + q
                            nc.tensor.transpose(
                                xtp[:, q * 128:(q + 1) * 128],
                                xg[:, tk * EP + kb * 128:
                                   tk * EP + (kb + 1) * 128],
                                ident_bf[:])
                        # xt col = kb*NTOK + s*64 + b  (s-major scan order)
                        if kb % 2 == 0:
                            nc.vector.tensor_copy(
                                out=xt[:, kb * NTOK + grp * 512:
                                       kb * NTOK + (grp + 1) * 512],
                                in_=xtp[:])
                        else:
                            nc.scalar.copy(
                                out=xt[:, kb * NTOK + grp * 512:
                                       kb * NTOK + (grp + 1) * 512],
                                in_=xtp[:])
                    items.append(transp)
                return items

            _nproj = [0]

            def emit_proj(ch, n):
                def proj():
                    zxp = ppool.tile([128, 512], F32, tag="zxp")
                    for kb in range(3):
                        nc.tensor.matmul(
                            zxp[:],
                            lhsT=wih_sb[:, (ch * 3 + kb) * 128:
                                        (ch * 3 + kb + 1) * 128],
                            rhs=xt[:, kb * NTOK + n * 512:
                                   kb * NTOK + (n + 1) * 512],
                            start=(kb == 0), stop=(kb == 2),
                        )
                    dst = zx[:, ch * 4096 + n * 512:ch * 4096 + (n + 1) * 512]
                    if _nproj[0] % 2 == 0:
                        nc.vector.tensor_copy(out=dst, in_=zxp[:])
                    else:
                        nc.scalar.copy(out=dst, in_=zxp[:])
                    _nproj[0] += 1
                return proj

            # pre-warm the PE during the gather window so projection and
            # the early recurrence run at K=8/8 (inputs land via DMA first)
            for _ in range(6):
                warm = ppool.tile([128, 512], F32, tag="zxp")
                nc.tensor.matmul(warm[:], lhsT=wstat_sb[:, 0:128],
                                 rhs=wstat_sb[:, 512:1024],
                                 start=True, stop=True)
            # prologue: gathers for groups 0-1 fired first (ahead of the
            # identity/memset gpsimd work in FIFO via deferred emission),
            # then transposes and projection n=0
            g0 = emit_group(0)
            g1 = emit_group(1)
            g0[0]()
            g1[0]()
            for it in g0[1:]:
                it()
            for ch in range(8):
                emit_proj(ch, 0)()
            for it in g1[1:]:
                it()
            # deferred work queue: proj n=1 then (group n, proj n) for
            # n = 2..7, drained at 3 items/step (large producer->consumer
            # slack; tighter just-in-time pacing exposed a missing-dep race)
            work = []
            for ch in range(8):
                work.append(emit_proj(ch, 1))
            for n in range(2, 8):
                work.extend(emit_group(n))
                for ch in range(8):
                    work.append(emit_proj(ch, n))

            # ---------- Phase B: recurrence ----------
            zx_v = zx[:].rearrange("p (c s b) -> p c s b", c=8, s=T)

            def pairs(zq, ch0, nch, s):
                for j in range(nch):
                    ch = ch0 + j
                    for k in range(2):
                        w_ap = wstat_sb[:, (ch * 2 + k) * 128:
                                        (ch * 2 + k + 1) * 128]
                        if s == 0:
                            rhs = hzero[:]
                        else:
                            rhs = h_all[:, (s - 1) * 128 + k * 64:
                                        (s - 1) * 128 + (k + 1) * 64]
                        nc.tensor.matmul(
                            zq[:, j * 64:(j + 1) * 64],
                            lhsT=w_ap, rhs=rhs,
                            start=False, stop=(k == 1),
                        )

            for s in range(T):
                # zx preloads (identity stationary; start clears the bank)
                nc.tensor.matmul(zq_if[:, 0:256], lhsT=ident_bf[:],
                                 rhs=zx_v[:, 0:4, s, :],
                                 start=True, stop=False)
                pairs(zq_if, 0, 4, s)
                nc.tensor.matmul(zq_g[:, 0:128], lhsT=ident_bf[:],
                                 rhs=zx_v[:, 4:6, s, :],
                                 start=True, stop=False)
                pairs(zq_g, 4, 2, s)
                nc.tensor.matmul(zq_o[:, 0:128], lhsT=ident_bf[:],
                                 rhs=zx_v[:, 6:8, s, :],
                                 start=True, stop=False)
                pairs(zq_o, 6, 2, s)

                nc.scalar.activation(sgif[:], zq_if[:, 0:256], AF.Sigmoid)
                nc.scalar.activation(g_t[:], zq_g[:, 0:128], AF.Tanh)
                nc.scalar.activation(o_t[:], zq_o[:, 0:128], AF.Sigmoid)
                nc.vector.tensor_mul(c_st[:], c_st[:], sgif[:, 128:256])
                nc.vector.tensor_mul(tmp_ig[:], sgif[:, 0:128], g_t[:])
                nc.vector.tensor_add(c_st[:], c_st[:], tmp_ig[:])
                nc.scalar.activation(tch[:], c_st[:], AF.Tanh)
                hslot = h_all[:, s * 128:(s + 1) * 128]
                nc.vector.tensor_mul(hslot, o_t[:], tch[:])
                # running masked max, off the h critical path
                nc.vector.tensor_add(hm_t[:], hslot,
                                     mbig_sb[:, s * 128:(s + 1) * 128])
                nc.vector.tensor_max(hmax[:], hmax[:], hm_t[:])

                # stream phase A work into the PE's elementwise stall;
                # once drained, top up with dummy matmuls into the zxp
                # ring so HAM never re-throttles (no extra PSUM bank)
                if work and s >= 2:
                    for _ in range(3):
                        if work:
                            work.pop(0)()
                elif s < T - 2:
                    for _ in range(3):
                        warm = ppool.tile([128, 512], F32, tag="zxp")
                        nc.tensor.matmul(warm[:], lhsT=wih_sb[:, 0:128],
                                         rhs=xt[:, 0:512],
                                         start=True, stop=True)


            # ---------- Phase C: transpose + output ----------
            tp = ppool.tile([128, 512], F32, tag="zxp")
            nc.tensor.transpose(tp[:, 0:128], hmax[:], ident[:])
            nc.vector.tensor_copy(out=hmaxT[:], in_=tp[:, 0:128])
            # out[b, k*128 + p] <- hmaxT[j = k*64 + b, p]
            out_ap = bass.AP(tensor=out_d[:, :].tensor, offset=0,
                             ap=[[128, 2], [HID, NSC], [1, 128]])
            nc.sync.dma_start(out=out_ap, in_=hmaxT[:])

    nc.finalize()
    return nc


def _host_prep(token_ids, lengths, emb, w_ih_f, w_hh_f, b_f, w_ih_b, w_hh_b,
               b_b):
    emb384 = np.zeros((V, EP), dtype=bf)
    emb384[:, :E] = emb.astype(bf)
    emb384[:, 300] = bf(1.0)            # bias lane rides the gather

    wstat_d, wih_d = {}, {}
    for d in range(2):
        whh = w_hh_f if d == 0 else w_hh_b
        wstat = np.zeros((128, 2048), dtype=bf)
        for ch in range(8):
            gb = GB_BASE[ch]
            for k in range(2):
                blk = whh[gb:gb + 128, k * 128:(k + 1) * 128].T
                col = (ch * 2 + k) * 128
                wstat[:, col:col + 128] = blk.astype(bf)
        wstat_d[d] = wstat

        w_ih = w_ih_f if d == 0 else w_ih_b
        bias = b_f if d == 0 else b_b
        aug = np.zeros((EP, 4 * HID), dtype=np.float32)
        aug[:E, :] = w_ih.T
        aug[300, :] = bias
        if d == 1:
            mv = np.zeros(4 * HID, dtype=np.float32)
            mv[0:512] = BIGNEG          # i, f
            mv[768:1024] = BIGNEG       # o
            aug[301, :] = mv
        wih = np.zeros((128, 3072), dtype=bf)
        for ch in range(8):
            gb = GB_BASE[ch]
            for kb in range(3):
                blk = aug[kb * 128:(kb + 1) * 128, gb:gb + 128]
                col = (ch * 3 + kb) * 128
                wih[:, col:col + 128] = blk.astype(bf)
        wih_d[d] = wih

    in_maps = []
    for c in range(NCORES):
        d = 0 if c < 4 else 1
        blk = c % 4
        tok = token_ids[blk * NSC:(blk + 1) * NSC]      # [64, 64]
        ln = lengths[blk * NSC:(blk + 1) * NSC]         # [64]
        if d == 1:
            tok = tok[:, ::-1]                          # scan order = reversed

        # gather tile tk holds tokens (s = 2*tk + p//64, b = p%64)
        tok_sm = tok.T.reshape(NTT, 128)                # [s, b] -> tiles
        idx = tok_sm.T.astype(np.int32).copy()          # [128, NTT]

        ss = np.arange(T)[None, :]
        t_of_s = ss if d == 0 else T - 1 - ss
        pad = (t_of_s >= ln[:, None]).astype(np.float32)   # [b, s] by scan s
        mflag = pad.T.reshape(NTT, 128).T.astype(bf).copy()

        # mbig[p, s*128 + k*64 + b] = MAXNEG where padded (all p, both k)
        mb_ = np.zeros((T, 2, NSC), dtype=np.float32)
        mb_[:, :, :] = np.where(pad.T, MAXNEG, 0.0)[:, None, :]
        mb_ = np.broadcast_to(mb_.reshape(1, T * 128), (128, T * 128))
        in_maps.append({
            "emb": emb384,
            "idx": idx,
            "mflag": mflag,
            "wstat": wstat_d[d],
            "wih": wih_d[d],
            "mbig": mb_.astype(bf),
        })
    return in_maps


def kernel(token_ids, lengths, emb, w_ih_f, w_hh_f, b_f, w_ih_b, w_hh_b, b_b):
    global LAST_RESULTS
    if "nc" not in _CACHE:
        _CACHE["nc"] = _build_program()
    nc = _CACHE["nc"]
    in_maps = _host_prep(token_ids, lengths, emb, w_ih_f, w_hh_f, b_f,
                         w_ih_b, w_hh_b, b_b)
    res = bass_utils.run_bass_kernel_spmd(nc, in_maps, list(range(NCORES)))
    LAST_RESULTS = res
    out = np.zeros((B, 2 * HID), np.float32)
    for c in range(NCORES):
        d = 0 if c < 4 else 1
        blk = c % 4
        out[blk * NSC:(blk + 1) * NSC,
            d * HID:(d + 1) * HID] = res.results[c]["out"]
    return out
